# revision 10
# baseline (speedup 1.0000x reference)
"""Multi-head attention (B=4, L=1024, D=1024, H=16, DH=64) on 8 TRN2 NeuronCores.

Sharding: data-parallel over batch (4) x tensor-parallel over heads (2).
Core c = 2*b + t computes, for batch b, heads [t*8, (t+1)*8):
    QT = Wq_t^T X^T, KT = Wk_t^T X^T, V = Y Wv_t        (all bf16 matmuls)
    per head: S^T = K_h Q_h^T; P^T = exp(S^T/8);
              [ctx^T; rowsum] = [V_h | 1]^T P^T;  ctxn = ctx / rowsum
    O_partial = ctxn^T Wo_t                              (f32 out)
Host pre-transposes X/Y and casts everything to bf16; host sums the two
tensor-parallel partials per batch in f32.
"""

import numpy as np
import ml_dtypes

import concourse.bass as bass
import concourse.tile as tile
import concourse.mybir as mybir
from concourse import bacc
from concourse.bass_utils import run_bass_kernel_spmd

B, L, D, U, H = 4, 1024, 1024, 1024, 16
DH = U // H          # 64 head dim
TP = 2               # tensor-parallel ways (heads)
DL = U // TP         # 512 local units
HL = H // TP         # 8 local heads
P = 128              # partitions
NI = 512             # matmul free-dim chunk (one PSUM bank of f32)
CC = D // P          # 8 contraction chunks for projections
DT = DL // P         # 4 local d-tiles
IT = L // P          # 8 i/j tiles
NIC = L // NI        # 2 free chunks of 512
N_CORES = 8

BF16 = mybir.dt.bfloat16
F32 = mybir.dt.float32


def _build_kernel():
    nc = bacc.Bacc(
        "TRN2", target_bir_lowering=False, debug=False, num_devices=N_CORES
    )
    xt = nc.dram_tensor("xt", [D, L], BF16, kind="ExternalInput").ap()
    yt = nc.dram_tensor("yt", [D, L], BF16, kind="ExternalInput").ap()
    wq = nc.dram_tensor("wq", [D, DL], BF16, kind="ExternalInput").ap()
    wk = nc.dram_tensor("wk", [D, DL], BF16, kind="ExternalInput").ap()
    wv = nc.dram_tensor("wv", [D, DL], BF16, kind="ExternalInput").ap()
    wo = nc.dram_tensor("wo", [DL, U], BF16, kind="ExternalInput").ap()
    out = nc.dram_tensor("out", [L, U], F32, kind="ExternalOutput").ap()

    with tile.TileContext(nc) as tc:
        _mha_body(tc, out, xt, yt, wq, wk, wv, wo)

    nc.compile()
    return nc


def _mha_body(tc, out, xt, yt, wq, wk, wv, wo, dbg=None):
    nc = tc.nc
    from contextlib import ExitStack

    with ExitStack() as ctx:
        persist = ctx.enter_context(tc.tile_pool(name="persist", bufs=1))
        pt_pool = ctx.enter_context(tc.tile_pool(name="pt", bufs=3))
        ps_acc = ctx.enter_context(tc.tile_pool(name="ps_acc", bufs=3, space="PSUM"))
        ps_st = ctx.enter_context(tc.tile_pool(name="ps_st", bufs=4, space="PSUM"))
        small = ctx.enter_context(tc.tile_pool(name="small", bufs=4))

        # persistent SBUF tensors
        xt_sb = persist.tile([P, CC, L], BF16, tag="xt")
        yt_sb = persist.tile([P, CC, L], BF16, tag="yt")
        wq_sb = persist.tile([P, CC, DL], BF16, tag="wq")
        wk_sb = persist.tile([P, CC, DL], BF16, tag="wk")
        wv_sb = persist.tile([P, CC, DL], BF16, tag="wv")
        wo_sb = persist.tile([P, DT, U], BF16, tag="wo")
        qt_sb = persist.tile([P, DT, L], BF16, tag="qt")
        kt_sb = persist.tile([P, DT, L], BF16, tag="kt")
        # Vaug: per j-chunk, per head a 128-col block: [V_h (64) | ones (64)]
        va_sb = persist.tile([P, IT, HL * P], BF16, tag="va")
        cx_sb = persist.tile([P, DT, L], BF16, tag="cx")

        # input DMAs (order roughly matches consumption)
        nc.sync.dma_start(out=wq_sb[:], in_=wq.rearrange("(cc p) d -> p cc d", p=P))
        nc.sync.dma_start(out=xt_sb[:], in_=xt.rearrange("(cc p) i -> p cc i", p=P))
        nc.sync.dma_start(out=wk_sb[:], in_=wk.rearrange("(cc p) d -> p cc d", p=P))
        nc.sync.dma_start(out=yt_sb[:], in_=yt.rearrange("(cc p) i -> p cc i", p=P))
        nc.sync.dma_start(out=wv_sb[:], in_=wv.rearrange("(cc p) d -> p cc d", p=P))
        nc.sync.dma_start(out=wo_sb[:], in_=wo.rearrange("(dt p) o -> p dt o", p=P))

        # ones columns of Vaug (V copies below overwrite the V halves)
        nc.vector.memset(va_sb[:], 1.0)

        # ---- projections ----
        # QT[d, i] and KT[d, i]: lhsT = W chunk, rhs = XT/YT chunk
        for w_sb, t_sb, rhs_sb in ((wq_sb, qt_sb, xt_sb), (wk_sb, kt_sb, yt_sb)):
            for dt in range(DT):
                for ic in range(NIC):
                    ps = ps_acc.tile([P, NI], F32, tag="acc")
                    for cc in range(CC):
                        nc.tensor.matmul(
                            ps[:],
                            w_sb[:, cc, dt * P : (dt + 1) * P],
                            rhs_sb[:, cc, ic * NI : (ic + 1) * NI],
                            start=(cc == 0),
                            stop=(cc == CC - 1),
                        )
                    nc.vector.tensor_copy(
                        t_sb[:, dt, ic * NI : (ic + 1) * NI], ps[:]
                    )
        # V[j, d]: lhsT = YT chunk (j block), rhs = Wv
        # Vaug block for head h: even h -> [V_h | ones], odd h -> [ones | V_h]
        # so that ctx^T lands on the same partitions the head's cx rows use.
        for jt in range(IT):
            ps = ps_acc.tile([P, NI], F32, tag="acc")
            for cc in range(CC):
                nc.tensor.matmul(
                    ps[:],
                    yt_sb[:, cc, jt * P : (jt + 1) * P],
                    wv_sb[:, cc, :],
                    start=(cc == 0),
                    stop=(cc == CC - 1),
                )
            va_blk = va_sb[:, jt].rearrange("p (h s) -> p h s", s=P)
            ps_blk = ps.rearrange("p (h s) -> p h s", s=DH)
            # even heads: V -> cols 0:64 of their block
            nc.vector.tensor_copy(va_blk[:, 0::2, 0:DH], ps_blk[:, 0::2, :])
            # odd heads: V -> cols 64:128 of their block
            nc.vector.tensor_copy(va_blk[:, 1::2, DH:P], ps_blk[:, 1::2, :])

        if dbg is not None:
            d_qt, d_kt, d_va, d_pt, d_cx = dbg
            nc.sync.dma_start(out=d_qt[:], in_=qt_sb[:])
            nc.sync.dma_start(out=d_kt[:], in_=kt_sb[:])
            nc.sync.dma_start(out=d_va[:], in_=va_sb[:])

        # ---- attention, head pairs share the PE via disjoint row groups ----
        scale = DH**-0.5
        for h in range(HL):
            dt, r0 = divmod(h * DH, P)
            ptile = pt_pool.tile([P, IT, L], BF16, tag="pt")
            for jt in range(IT):
                for ic in range(NIC):
                    st = ps_st.tile([P, NI], F32, tag="st")
                    nc.tensor.matmul(
                        st[:],
                        kt_sb[r0 : r0 + DH, dt, jt * P : (jt + 1) * P],
                        qt_sb[r0 : r0 + DH, dt, ic * NI : (ic + 1) * NI],
                        start=True,
                        stop=True,
                    )
                    nc.scalar.activation(
                        ptile[:, jt, ic * NI : (ic + 1) * NI],
                        st[:],
                        mybir.ActivationFunctionType.Exp,
                        scale=scale,
                    )
            if dbg is not None and h < 2:
                nc.sync.dma_start(out=d_pt[h], in_=ptile[:])
            for ic in range(NIC):
                ct = ps_acc.tile([P, NI], F32, tag="acc")
                for jt in range(IT):
                    nc.tensor.matmul(
                        ct[:],
                        va_sb[:, jt, h * P : (h + 1) * P],
                        ptile[:, jt, ic * NI : (ic + 1) * NI],
                        start=(jt == 0),
                        stop=(jt == IT - 1),
                    )
                # ctx^T sits on partitions [r0, r0+64); rowsum on the other
                # half. DVE ops must be partition-aligned, and custom DVE ops
                # (reciprocal) only work at base partition 0; cross-partition
                # moves go through small SBUF->SBUF DMAs.
                rc = small.tile([P, NI], F32, tag="rc")
                if r0 == 0:
                    # rowsum on [64:128): copy @64, shift down, recip @0
                    rs = small.tile([P, NI], F32, tag="rs")
                    nc.vector.tensor_copy(rs[DH:P, :], ct[DH:P, :])
                    nc.sync.dma_start(out=rs[0:DH, :], in_=rs[DH:P, :])
                    nc.vector.reciprocal_approx_fast(rc[0:DH, :], rs[0:DH, :])
                else:
                    # rowsum on [0:64): recip @0, shift up
                    nc.vector.reciprocal_approx_fast(rc[0:DH, :], ct[0:DH, :])
                    nc.sync.dma_start(out=rc[DH:P, :], in_=rc[0:DH, :])
                nc.vector.tensor_mul(
                    cx_sb[r0 : r0 + DH, dt, ic * NI : (ic + 1) * NI],
                    ct[r0 : r0 + DH, :],
                    rc[r0 : r0 + DH, :],
                )

        if dbg is not None:
            nc.sync.dma_start(out=d_cx[:], in_=cx_sb[:])

        # ---- output projection ----
        out_r = out.rearrange("(it p) o -> it p o", p=P)
        for it in range(IT):
            for oc in range(NIC):
                po = ps_acc.tile([P, NI], F32, tag="acc")
                for dt in range(DT):
                    nc.tensor.matmul(
                        po[:],
                        cx_sb[:, dt, it * P : (it + 1) * P],
                        wo_sb[:, dt, oc * NI : (oc + 1) * NI],
                        start=(dt == 0),
                        stop=(dt == DT - 1),
                    )
                o_st = small.tile([P, NI], F32, tag="ost")
                nc.vector.tensor_copy(o_st[:], po[:])
                nc.sync.dma_start(
                    out=out_r[it, :, oc * NI : (oc + 1) * NI], in_=o_st[:]
                )


_NC_CACHE = None


def _get_nc():
    global _NC_CACHE
    if _NC_CACHE is None:
        _NC_CACHE = _build_kernel()
    return _NC_CACHE


def kernel(x, y, Wq, Wk, Wv, Wo, _trace=False):
    bf = ml_dtypes.bfloat16
    x = np.asarray(x, np.float32)
    y = np.asarray(y, np.float32)
    xtb = [np.ascontiguousarray(np.asarray(x[b]).T).astype(bf) for b in range(B)]
    ytb = [np.ascontiguousarray(np.asarray(y[b]).T).astype(bf) for b in range(B)]
    wqs = [np.ascontiguousarray(np.asarray(Wq)[:, t * DL : (t + 1) * DL]).astype(bf) for t in range(TP)]
    wks = [np.ascontiguousarray(np.asarray(Wk)[:, t * DL : (t + 1) * DL]).astype(bf) for t in range(TP)]
    wvs = [np.ascontiguousarray(np.asarray(Wv)[:, t * DL : (t + 1) * DL]).astype(bf) for t in range(TP)]
    wos = [np.ascontiguousarray(np.asarray(Wo)[t * DL : (t + 1) * DL, :]).astype(bf) for t in range(TP)]

    in_maps = []
    for b in range(B):
        for t in range(TP):
            in_maps.append(
                {
                    "xt": xtb[b],
                    "yt": ytb[b],
                    "wq": wqs[t],
                    "wk": wks[t],
                    "wv": wvs[t],
                    "wo": wos[t],
                }
            )

    nc = _get_nc()
    res = run_bass_kernel_spmd(
        nc, in_maps, core_ids=list(range(N_CORES)), trace=_trace
    )
    out = np.empty((B, L, U), np.float32)
    for b in range(B):
        out[b] = res.results[2 * b]["out"] + res.results[2 * b + 1]["out"]
    if _trace:
        return out, res
    return out
